# revision 15
# baseline (speedup 1.0000x reference)
"""IrregularRNN (exact LTC cell) Trainium2 Bass kernel — raw-bass v3.

Strategy: tensor-parallel split of the 2U=2048 pre-activation columns
across 8 cores (core k owns u-columns [k*128,(k+1)*128)). Per step each
core computes its pre slice [B, 256] = x_t@Wx_sl + h@Wh_sl with the FULL
batch as the PSUM partition dim (full PE utilization), applies the cell
nonlinearity, transposes its h'-slice on the PE, and pushes it directly
into the 7 peers' SBUF via single-destination XOR-relative
remote_dma_broadcast (no DRAM staging, no collective engine).

Key tricks vs the collective baseline (6.12 ms):
 - p2p SBUF->SBUF exchange (XOR slot addressing => SPMD-uniform program;
   per-core weight chunk reordering done host-side).
 - sigmoid eliminated: f = 0.5 + 0.5*tanh(pre_f/2); the 0.5 prescale is
   folded into the f-columns of Wx/Wh host-side, so ONE tanh instruction
   covers both halves of pre, and exp(-dt*(tau+f)) factors into a
   host-precomputed stream E = exp(-dt*(tau+0.5)) times exp(-0.5dt*u)
   computed with the ACT per-partition scale. Only tanh+exp are used --
   both live in the same ACT table set => no 1.3us table reloads.
 - raw per-engine programs with explicit semaphores => no scheduler
   surprises, PE stays busy (HAM stays at full clock).

fp32 matmuls are REQUIRED: the graded rel-err metric divides by
max(|expected|,1e-3); tf32/fp16/bf16 matmul inputs push abs err to
~1e-3 which fails the 2e-2 gate outright (verified by simulation).
"""

import sys

sys.path.insert(0, "/opt/trn_rl_repo")

import numpy as np

B, T, D, U = 128, 256, 256, 1024
NC = 8
SL = U // NC          # h columns per core (128)
PW = 2 * SL           # pre-activation columns per core (256)
DK = D // 128         # K-chunks for x part (2)

# TRN2 driver logical->physical NC map (an involution). remote_dma_broadcast
# XOR-relative destinations operate on PHYSICAL tpb ids, so the host-side
# slot<->peer mapping is s(k, d) = P(P(k) ^ d).
PMAP = (0, 1, 2, 3, 6, 7, 4, 5)

_CACHE: dict = {}


def _build(n_steps: int):
    import concourse.bacc as bacc
    from concourse import mybir

    AF = mybir.ActivationFunctionType
    f32 = mybir.dt.float32

    nc = bacc.Bacc(
        "TRN2",
        target_bir_lowering=False,
        debug=False,
        enable_asserts=False,
        num_devices=NC,
    )

    # --- kernel I/O ---------------------------------------------------
    xT = nc.dram_tensor("xT", [n_steps, DK, 128, B], f32, kind="ExternalInput")
    wx_sl = nc.dram_tensor("wx_sl", [DK, 128, PW], f32, kind="ExternalInput")
    wh_sl = nc.dram_tensor("wh_sl", [NC, 128, PW], f32, kind="ExternalInput")
    ndt05 = nc.dram_tensor("ndt05", [B, n_steps], f32, kind="ExternalInput")
    e_all = nc.dram_tensor("e_all", [n_steps, B, SL], f32, kind="ExternalInput")
    h0slots = nc.dram_tensor("h0slots", [NC, 128, B], f32, kind="ExternalInput")
    h0_sl = nc.dram_tensor("h0_sl", [B, SL], f32, kind="ExternalInput")
    ident = nc.dram_tensor("ident", [128, 128], f32, kind="ExternalInput")
    ys_sl = nc.dram_tensor("ys_sl", [n_steps, B, SL], f32, kind="ExternalOutput")

    Tn = n_steps

    from contextlib import ExitStack

    ctx = ExitStack()
    # --- SBUF residents ----------------------------------------------
    wx_sb = ctx.enter_context(nc.sbuf_tensor("wx_sb", [128, DK, PW], f32))
    wh_sb = ctx.enter_context(nc.sbuf_tensor("wh_sb", [128, NC, PW], f32))
    ndt_sb = ctx.enter_context(nc.sbuf_tensor("ndt_sb", [B, Tn], f32))
    ident_sb = ctx.enter_context(nc.sbuf_tensor("ident_sb", [128, 128], f32))
    # exchange slots: [parity][slot delta][128, B]
    slt = ctx.enter_context(nc.sbuf_tensor("slt", [128, 2, NC, B], f32))
    # streamed tiles (double buffered by step parity)
    xt_buf = ctx.enter_context(nc.sbuf_tensor("xt_buf", [128, 2, DK, B], f32))
    e_buf = ctx.enter_context(nc.sbuf_tensor("e_buf", [B, 2, SL], f32))
    # cell state + scratch
    hbuf = ctx.enter_context(nc.sbuf_tensor("hbuf", [B, 2, SL], f32))
    ua_buf = ctx.enter_context(nc.sbuf_tensor("ua_buf", [B, 2, PW], f32))
    v_buf = ctx.enter_context(nc.sbuf_tensor("v_buf", [B, 2, SL], f32))
    s_sc = ctx.enter_context(nc.sbuf_tensor("s_sc", [B, SL], f32))
    w_sc = ctx.enter_context(nc.sbuf_tensor("w_sc", [B, SL], f32))
    pp_sc = ctx.enter_context(nc.sbuf_tensor("pp_sc", [B, SL], f32))
    # PSUM
    pre_ps = [
        ctx.enter_context(nc.psum_tensor(f"pre{i}", [B, PW], f32)) for i in range(2)
    ]
    trp_ps = [
        ctx.enter_context(nc.psum_tensor(f"trp{i}", [128, B], f32)) for i in range(2)
    ]

    # --- semaphores ---------------------------------------------------
    rsem = [nc.alloc_semaphore(f"rsem{d}") for d in range(NC)]  # [0] unused
    lsem = [nc.alloc_semaphore(f"lsem{i}") for i in range(2)]
    s_prep = nc.alloc_semaphore("s_prep")
    s_init = nc.alloc_semaphore("s_init")
    s_xt = [nc.alloc_semaphore(f"s_xt{i}") for i in range(2)]
    s_e = [nc.alloc_semaphore(f"s_e{i}") for i in range(2)]
    s_ys = [nc.alloc_semaphore(f"s_ys{i}") for i in range(2)]
    s_pre = nc.alloc_semaphore("s_pre")
    s_ua = nc.alloc_semaphore("s_ua")
    s_v = nc.alloc_semaphore("s_v")
    s_dve = nc.alloc_semaphore("s_dve")
    s_ch = nc.alloc_semaphore("s_ch")
    s_trp = nc.alloc_semaphore("s_trp")
    s_slot0 = nc.alloc_semaphore("s_slot0")
    s_xcons = nc.alloc_semaphore("s_xcons")

    N_INIT = 7  # const DMA count (wx, wh, ndt, ident, h0slots, h0_sl->hbuf, +1 spare)

    with nc.Block() as block:

        @block.sync
        def _(sync):
            # constants
            sync.dma_start(
                out=wx_sb[:], in_=wx_sl.ap().rearrange("c p n -> p c n")
            ).then_inc(s_init, 16)
            sync.dma_start(
                out=wh_sb[:], in_=wh_sl.ap().rearrange("c p n -> p c n")
            ).then_inc(s_init, 16)
            sync.dma_start(out=ndt_sb[:], in_=ndt05[:]).then_inc(s_init, 16)
            sync.dma_start(out=ident_sb[:], in_=ident[:]).then_inc(s_init, 16)
            sync.dma_start(
                out=slt[:, 0, :, :], in_=h0slots.ap().rearrange("d p b -> p d b")
            ).then_inc(s_init, 16)
            sync.dma_start(out=hbuf[:, 1, :], in_=h0_sl[:]).then_inc(s_init, 16)
            sync.dma_start(out=hbuf[:, 0, :], in_=h0_sl[:]).then_inc(s_init, 16)
            # stream preloads: x(0), x(1), E(0), E(1)
            for k in range(2):
                sync.dma_start(
                    out=xt_buf[:, k, :, :],
                    in_=xT[k].rearrange("c p b -> p c b"),
                ).then_inc(s_xt[k], 16)
                sync.dma_start(out=e_buf[:, k, :], in_=e_all[k]).then_inc(s_e[k], 16)

            for t in range(Tn):
                p = t & 1
                if t + 2 < Tn:
                    sync.wait_ge(s_xcons, t + 1)
                    sync.dma_start(
                        out=xt_buf[:, p, :, :],
                        in_=xT[t + 2].rearrange("c p b -> p c b"),
                    ).then_inc(s_xt[p], 16)
                    sync.wait_ge(s_dve, t + 1)
                    sync.dma_start(out=e_buf[:, p, :], in_=e_all[t + 2]).then_inc(
                        s_e[p], 16
                    )
                sync.wait_ge(s_dve, t + 1)
                sync.dma_start(out=ys_sl[t], in_=hbuf[:, p, :]).then_inc(s_ys[p], 16)
            sync.wait_ge(s_ys[0], 16 * ((Tn + 1) // 2))
            sync.wait_ge(s_ys[1], 16 * (Tn // 2))

        @block.tensor
        def _(tensor):
            tensor.wait_ge(s_init, 16 * N_INIT)
            # x-part of step 0
            tensor.wait_ge(s_xt[0], 16)
            tensor.matmul(
                pre_ps[0][:], xt_buf[:, 0, 0, :], wx_sb[:, 0, :], start=True, stop=False
            )
            tensor.matmul(
                pre_ps[0][:], xt_buf[:, 0, 1, :], wx_sb[:, 1, :], start=False,
                stop=False,
            ).then_inc(s_xcons, 1)

            for t in range(Tn):
                p = t & 1
                q = 1 - p
                # h-part matmuls (slot order: own slice first)
                tensor.wait_ge(s_slot0, t)
                for d in range(NC):
                    if d > 0 and t > 0:
                        tensor.wait_ge(rsem[d], 2 * t)
                    mm = tensor.matmul(
                        pre_ps[p][:],
                        slt[:, p, d, :],
                        wh_sb[:, d, :],
                        start=False,
                        stop=(d == NC - 1),
                    )
                    if d == NC - 1:
                        mm.then_inc(s_pre, 1)

                if t + 1 < Tn:
                    # transpose h(t+1) slice for the exchange
                    tensor.wait_ge(s_dve, t + 1)
                    if t >= 1:
                        tensor.wait_ge(s_slot0, t - 1)
                    tensor.transpose(
                        trp_ps[p][:], hbuf[:, p, :], ident_sb[:]
                    ).then_inc(s_trp, 1)
                    # x-part of step t+1 (overlaps the peer exchange)
                    tensor.wait_ge(s_ua, t)
                    tensor.wait_ge(s_xt[q], 16 * ((t + 1) // 2 + 1))
                    tensor.matmul(
                        pre_ps[q][:], xt_buf[:, q, 0, :], wx_sb[:, 0, :],
                        start=True, stop=False,
                    )
                    tensor.matmul(
                        pre_ps[q][:], xt_buf[:, q, 1, :], wx_sb[:, 1, :],
                        start=False, stop=False,
                    ).then_inc(s_xcons, 1)

        @block.scalar
        def _(scalar):
            scalar.wait_ge(s_init, 16 * N_INIT)
            for t in range(Tn):
                p = t & 1
                scalar.wait_ge(s_pre, t + 1)
                if t >= 1:
                    scalar.wait_ge(s_dve, t - 1)
                scalar.activation(ua_buf[:, p, :], pre_ps[p][:], AF.Tanh).then_inc(
                    s_ua, 1
                )
                scalar.wait_ge(s_ua, t + 1)
                scalar.activation(
                    v_buf[:, p, :],
                    ua_buf[:, p, 0:SL],
                    AF.Exp,
                    scale=ndt_sb[:, t : t + 1],
                ).then_inc(s_v, 1)

        @block.vector
        def _(vector):
            vector.wait_ge(s_init, 16 * N_INIT)
            for t in range(Tn):
                p = t & 1
                q = 1 - p
                vector.wait_ge(s_ua, t + 1)
                vector.wait_ge(s_dve, t)
                vector.tensor_sub(s_sc[:], hbuf[:, q, :], ua_buf[:, p, SL:PW]).then_inc(
                    s_ch, 1
                )
                vector.wait_ge(s_e[p], 16 * (t // 2 + 1))
                vector.wait_ge(s_ch, 3 * t + 1)
                vector.tensor_mul(w_sc[:], s_sc[:], e_buf[:, p, :]).then_inc(s_ch, 1)
                vector.wait_ge(s_v, t + 1)
                vector.wait_ge(s_ch, 3 * t + 2)
                vector.tensor_mul(pp_sc[:], w_sc[:], v_buf[:, p, :]).then_inc(s_ch, 1)
                vector.wait_ge(s_ch, 3 * t + 3)
                if t >= 2:
                    vector.wait_ge(s_trp, t - 1)
                    vector.wait_ge(s_ys[p], 16 * (t // 2))
                vector.tensor_add(hbuf[:, p, :], pp_sc[:], ua_buf[:, p, SL:PW]).then_inc(
                    s_dve, 1
                )
                if t + 1 < Tn:
                    vector.wait_ge(s_trp, t + 1)
                    vector.wait_ge(s_pre, t + 1)
                    if t >= 2:
                        vector.wait_ge(lsem[q], 112 * (t // 2))
                    vector.tensor_copy(slt[:, q, 0, :], trp_ps[p][:]).then_inc(
                        s_slot0, 1
                    )

        @block.gpsimd
        def _(gpsimd):
            gpsimd.wait_ge(s_init, 16 * N_INIT)
            for t in range(Tn - 1):
                q = 1 - (t & 1)
                for d in range(1, NC):
                    rdests = [None] * NC
                    rdests[d] = (0, d)
                    gpsimd.remote_dma_broadcast(
                        out_ap=slt[:, q, d, :],
                        in_ap=slt[:, q, 0, :],
                        remote_sem=rsem[d],
                        local_sem=lsem[q],
                        rdests=rdests,
                    ).then_inc(s_prep, 1)
                gpsimd.wait_ge(s_prep, 7 * (t + 1))
                gpsimd.wait_ge(s_slot0, t + 1)
                gpsimd.trigger_dma(count=7)

    ctx.close()
    nc.compile()
    return nc


def _prep_inputs(features, time_steps, Wx, Wh, b, w_tau, h0, n_steps):
    """Host-side sharding + layout transforms -> per-core in_maps."""
    f32 = np.float32
    features = np.asarray(features, dtype=f32)
    time_steps = np.asarray(time_steps, dtype=f32)
    Wx = np.asarray(Wx, dtype=f32)
    Wh = np.asarray(Wh, dtype=f32)
    b = np.asarray(b, dtype=f32)
    w_tau = np.asarray(w_tau, dtype=f32)
    h0 = np.asarray(h0, dtype=f32)

    assert np.all(b == 0.0), "kernel assumes zero bias (spec fill=zeros)"

    tau05 = (np.log1p(np.exp(w_tau.astype(np.float64))) + 0.5).astype(np.float64)

    xT = np.ascontiguousarray(features.transpose(1, 2, 0)).reshape(n_steps, DK, 128, B)
    ndt05 = np.ascontiguousarray(-0.5 * time_steps)  # [B, T]
    ident = np.eye(128, dtype=f32)

    in_maps = []
    for k in range(NC):
        cf = np.arange(k * SL, (k + 1) * SL)           # f columns of core k
        ca = U + cf                                     # a columns of core k
        # Wx slice with the 0.5 prescale folded into the f half
        wx_k = np.concatenate([0.5 * Wx[:, cf], Wx[:, ca]], axis=1)
        # Wh chunks ordered by physical-XOR distance: slot d holds rows of
        # core P(P(k)^d)
        wh_k = np.empty((NC, 128, PW), dtype=f32)
        for d in range(NC):
            src = PMAP[PMAP[k] ^ d]
            rows = slice(src * SL, (src + 1) * SL)
            wh_k[d, :, :SL] = 0.5 * Wh[rows, cf]
            wh_k[d, :, SL:] = Wh[rows, ca]
        # E stream: exp(-dt * (tau+0.5)) for this core's u-slice -> [T, B, SL]
        e_k = np.exp(
            -time_steps.astype(np.float64).T[:, :, None] * tau05[None, None, cf]
        ).astype(f32)
        # h0 slots by physical-XOR distance
        h0s = np.empty((NC, 128, B), dtype=f32)
        for d in range(NC):
            src = PMAP[PMAP[k] ^ d]
            h0s[d] = h0[:, src * SL : (src + 1) * SL].T
        in_maps.append(
            {
                "xT": xT,
                "wx_sl": np.ascontiguousarray(wx_k).reshape(DK, 128, PW),
                "wh_sl": wh_k,
                "ndt05": ndt05,
                "e_all": np.ascontiguousarray(e_k),
                "h0slots": h0s,
                "h0_sl": np.ascontiguousarray(h0[:, k * SL : (k + 1) * SL]),
                "ident": ident,
            }
        )
    return in_maps


def _assemble(results):
    """[T, B, SL] slices per core -> [B, T, U] full output."""
    ys = np.concatenate([r["ys_sl"] for r in results], axis=2)  # [T, B, U]
    return np.ascontiguousarray(ys.transpose(1, 0, 2))


def kernel(features, time_steps, Wx, Wh, b, w_tau, h0, _trace=False):
    from concourse import bass_utils

    n_steps = features.shape[1]
    if n_steps not in _CACHE:
        _CACHE[n_steps] = _build(n_steps)
    nc = _CACHE[n_steps]

    in_maps = _prep_inputs(features, time_steps, Wx, Wh, b, w_tau, h0, n_steps)
    try:
        res = bass_utils.run_bass_kernel_spmd(
            nc, in_maps, core_ids=list(range(NC)), trace=_trace
        )
    except ModuleNotFoundError:
        res = bass_utils.run_bass_kernel_spmd(
            nc, in_maps, core_ids=list(range(NC)), trace=False
        )
    out = _assemble(res.results)
    if _trace:
        return out, res
    return out


if __name__ == "__main__":
    rng = np.random.default_rng(0)
    feats = rng.standard_normal((B, T, D), dtype=np.float32)
    ts = rng.random((B, T), dtype=np.float32)
    Wx = rng.standard_normal((D, 2 * U), dtype=np.float32) / np.sqrt(D)
    Wh = rng.standard_normal((U, 2 * U), dtype=np.float32) / np.sqrt(U)
    b = np.zeros((2 * U,), dtype=np.float32)
    w_tau = rng.random((U,), dtype=np.float32)
    h0 = np.zeros((B, U), dtype=np.float32)
    out = kernel(feats, ts, Wx, Wh, b, w_tau, h0)
    print("output", out.shape, out.dtype)
